# revision 18
# baseline (speedup 1.0000x reference)
"""Trainium2 Bass kernel for ComplexDifferentialAttention.

Sharding: 96 (head, q-tile-of-128) units over 8 cores; each core gets
8 q-tiles of one head (A) + 4 q-tiles of another head (B), so the SPMD
program is identical on every core: 3 batches of 4 q-tiles with
head-slot pattern (A, A, B).

Engine plan (v2):
- PE stream: sc(0,0) | sc(0,1) av(0,0) | sc(1,0) av(0,1) ep(0) |
  sc(1,1) av(1,0) | sc(2,0) av(1,1) ep(1) | sc(2,1) av(2,0) | av(2,1)
  ep(2) -- kept dense so the PE pstate ramps to full clock.
- DVE: squares of score PSUM (drains banks), epilogue accums/scalars.
- GpSimd: half-adds (|s|^2), av drains, u/t builds, half the gating.
- Scalar: ONLY sqrt and exp, in per-branch blocks (sqrt x4-pairs then
  exp x4-pairs) so activation-table reloads drop to ~4/batch; rms and
  output copies slotted into matching table groups.
- Softmax row-sum (ones-column trick) and RMS normalization folded into
  per-query scalars c1 = rinv/sum1, c2 = -lam*rinv/sum2, batched
  [128,4]-wide across the 4 units of a batch.
- et and vp in bf16 (prob/value matmul), everything pre-exp in f32.
"""
import sys, os, math
sys.path.insert(0, '/opt/trn_rl_repo')
import numpy as np
from contextlib import ExitStack

import concourse.bacc as bacc
import concourse.tile as tile
from concourse import mybir
from concourse.bass_utils import run_bass_kernel_spmd
from concourse.masks import make_identity

F32 = mybir.dt.float32
F32R = mybir.dt.float32r
BF16 = mybir.dt.bfloat16
AF = mybir.ActivationFunctionType
OP = mybir.AluOpType

D = 128
S = 1024
H = 12
NCORES = 8
NB = 3          # batches per core, 4 units each
SCALE = 1.0 / math.sqrt(D)
LAMBDA_INIT = 0.8 - 0.6 * math.exp(-0.3)

_prog_cache = {}


def _core_units(c):
    """Units for core c: list of (head, qtile). 8 of head A + 4 of head B."""
    k, odd = divmod(c, 2)
    hA = 3 * k + odd          # cores 2k -> 3k ; 2k+1 -> 3k+1
    hB = 3 * k + 2
    qoff = 0 if odd == 0 else 4
    return [(hA, q) for q in range(8)] + [(hB, qoff + q) for q in range(4)]


def _build_program():
    nc = bacc.Bacc("TRN2", target_bir_lowering=False, debug=False,
                   num_devices=NCORES)

    def din(name, shape, dt=F32R):
        return nc.dram_tensor(name, shape, dt, kind="ExternalInput").ap()

    qT_r = din("qT_r", [128, 12 * 128]); qT_i = din("qT_i", [128, 12 * 128])
    peq_r = din("peq_r", [128, 12 * 128], F32)
    peq_i = din("peq_i", [128, 12 * 128], F32)
    kT_r = din("kT_r", [128, 2048]); kT_i = din("kT_i", [128, 2048])
    pek_r = din("pek_r", [128, 2048], F32); pek_i = din("pek_i", [128, 2048], F32)
    vT_r = din("vT_r", [128, 2048]); vT_i = din("vT_i", [128, 2048])
    wq_r = din("wq_r", [128, 256]); wq_i = din("wq_i", [128, 256])
    wq_in = din("wq_in", [128, 256])
    wk_r = din("wk_r", [128, 128]); wk_i = din("wk_i", [128, 128])
    wk_in = din("wk_in", [128, 128])
    wv1 = din("wv1", [128, 256]); wv2 = din("wv2", [128, 256])
    wg_r = din("wg_r", [128, 128]); wg_i = din("wg_i", [128, 128])
    wg_in = din("wg_in", [128, 128])
    wo1 = din("wo1", [128, 256]); wo2 = din("wo2", [128, 256])
    lamneg = din("lamneg", [128, 1], F32)
    out_d = nc.dram_tensor("out", [12 * 128, 256], F32, kind="ExternalOutput").ap()

    with tile.TileContext(nc) as tc, ExitStack() as ctx:
        cst = ctx.enter_context(tc.tile_pool(name="cst", bufs=1))
        qpp = ctx.enter_context(tc.tile_pool(name="qpp", bufs=1))
        kpp = ctx.enter_context(tc.tile_pool(name="kpp", bufs=1))
        vpp = ctx.enter_context(tc.tile_pool(name="vpp", bufs=1))
        hot = ctx.enter_context(tc.tile_pool(name="hot", bufs=1))
        epi = ctx.enter_context(tc.tile_pool(name="epi", bufs=2))
        osb = ctx.enter_context(tc.tile_pool(name="osb", bufs=2))
        scp = ctx.enter_context(tc.tile_pool(name="scp", bufs=2, space="PSUM"))
        avp = ctx.enter_context(tc.tile_pool(name="avp", bufs=4, space="PSUM"))
        # input pools closed right after their last use to free SBUF
        # (LIFO: qin sits on top of kin so it can close first)
        kinctx = ExitStack()
        kin = kinctx.enter_context(tc.tile_pool(name="kin", bufs=1))
        qinctx = ExitStack()
        qin = qinctx.enter_context(tc.tile_pool(name="qin", bufs=1))

        # ---- weights + constants (gpsimd DMA queue: ~25ns issue) ----
        W = {}
        for nm, ap, w in [("wq_r", wq_r, 256), ("wq_i", wq_i, 256),
                          ("wq_in", wq_in, 256), ("wk_r", wk_r, 128),
                          ("wk_i", wk_i, 128), ("wk_in", wk_in, 128),
                          ("wv1", wv1, 256), ("wv2", wv2, 256),
                          ("wg_r", wg_r, 128), ("wg_i", wg_i, 128),
                          ("wg_in", wg_in, 128), ("wo1", wo1, 256),
                          ("wo2", wo2, 256)]:
            t = cst.tile([128, w], F32R, name=f"w_{nm}", tag=f"w_{nm}")
            q = nc.sync if nm.startswith("wq") else nc.scalar
            q.dma_start(t[:], ap[:])
            W[nm] = t
        lam_t = cst.tile([128, 1], F32)
        nc.scalar.dma_start(lam_t[:], lamneg[:])
        ident = cst.tile([128, 128], F32)
        make_identity(nc, ident[:])
        eps8 = cst.tile([128, 1], F32)
        nc.vector.memset(eps8[:], 1e-8)
        eps5 = cst.tile([128, 1], F32)
        nc.vector.memset(eps5[:], 1e-5)

        # ---- q-side inputs (chunked DMAs so matmuls start early) ----
        qtr = qin.tile([128, 1536], F32R, name="qtr", tag="qtr")
        qti = qin.tile([128, 1536], F32R, name="qti", tag="qti")
        pqr = qin.tile([128, 1536], F32, name="pqr", tag="pqr")
        pqi = qin.tile([128, 1536], F32, name="pqi", tag="pqi")
        for ch in range(3):
            cs = slice(ch * 512, (ch + 1) * 512)
            nc.sync.dma_start(qtr[:, cs], qT_r[:, cs])
            nc.sync.dma_start(qti[:, cs], qT_i[:, cs])
            nc.scalar.dma_start(pqr[:, cs], peq_r[:, cs])
            nc.scalar.dma_start(pqi[:, cs], peq_i[:, cs])

        kint = {}
        for nm in ("ktr", "kti", "pkr", "pki", "vtr", "vti"):
            kint[nm] = kin.tile([128, 1024], F32 if nm in ("pkr", "pki")
                                else F32R, name=nm, tag=nm)

        def load_k_side(hs, q):
            ks_ = slice(hs * 1024, (hs + 1) * 1024)
            for nm, src in [("ktr", kT_r), ("kti", kT_i), ("vtr", vT_r),
                            ("vti", vT_i), ("pkr", pek_r), ("pki", pek_i)]:
                for ch in range(2):
                    cs = slice(ch * 512, (ch + 1) * 512)
                    gs = slice(hs * 1024 + ch * 512, hs * 1024 + (ch + 1) * 512)
                    q.dma_start(kint[nm][:, cs], src[:, gs])

        load_k_side(0, nc.gpsimd)

        # ---- q projection (+pe_q) ----
        qp = {}
        for half in (0, 1):
            hs_ = slice(half * 128, (half + 1) * 128)
            for part in ("r", "i"):
                t = qpp.tile([128, 1536], F32R, name=f"qp{half}{part}",
                             tag=f"qp{half}{part}")
                qp[(half, part)] = t
                for ch in range(3):
                    cs = slice(ch * 512, (ch + 1) * 512)
                    ps = scp.tile([128, 1024], F32, name="scps", tag="sc")
                    if part == "r":
                        nc.tensor.matmul(ps[:, 0:512], W["wq_r"][:, hs_],
                                         qtr[:, cs], start=True, stop=False)
                        nc.tensor.matmul(ps[:, 0:512], W["wq_in"][:, hs_],
                                         qti[:, cs], start=False, stop=True)
                        pe = pqr
                    else:
                        nc.tensor.matmul(ps[:, 0:512], W["wq_i"][:, hs_],
                                         qtr[:, cs], start=True, stop=False)
                        nc.tensor.matmul(ps[:, 0:512], W["wq_r"][:, hs_],
                                         qti[:, cs], start=False, stop=True)
                        pe = pqi
                    nc.vector.tensor_add(t[:, cs], ps[:, 0:512], pe[:, cs])

        # ---- K/V prep for a head slot ----
        kp = {}   # (hs, 'r'|'i'|'in') -> [o=128, k=1024] f32r
        vp = {}   # (hs, chunk) -> [k=128, 258] bf16  ([vp_r | vp_i | 1])

        def kv_prep(hs):
            for part in ("r", "i"):
                t = kpp.tile([128, 1024], F32R, name=f"kp{hs}{part}",
                             tag=f"kp{hs}{part}")
                kp[(hs, part)] = t
                for ch in range(2):
                    cs = slice(ch * 512, (ch + 1) * 512)
                    ps = scp.tile([128, 1024], F32, name="scps", tag="sc")
                    if part == "r":
                        nc.tensor.matmul(ps[:, 0:512], W["wk_r"][:],
                                         kint["ktr"][:, cs], start=True, stop=False)
                        nc.tensor.matmul(ps[:, 0:512], W["wk_in"][:],
                                         kint["kti"][:, cs], start=False, stop=True)
                        pe = kint["pkr"]
                    else:
                        nc.tensor.matmul(ps[:, 0:512], W["wk_i"][:],
                                         kint["ktr"][:, cs], start=True, stop=False)
                        nc.tensor.matmul(ps[:, 0:512], W["wk_r"][:],
                                         kint["kti"][:, cs], start=False, stop=True)
                        pe = kint["pki"]
                    nc.vector.tensor_add(t[:, cs], ps[:, 0:512], pe[:, cs])
            tn = kpp.tile([128, 1024], F32R, name=f"kp{hs}in", tag=f"kp{hs}in")
            kp[(hs, "in")] = tn
            nc.vector.tensor_scalar_mul(tn[:], kp[(hs, "i")][:], -1.0)
            for ch in range(8):
                cs = slice(ch * 128, (ch + 1) * 128)
                ps = avp.tile([128, 258], F32, name="vps", tag="av")
                nc.tensor.matmul(ps[:, 0:256], kint["vtr"][:, cs], W["wv1"][:],
                                 start=True, stop=False)
                nc.tensor.matmul(ps[:, 0:256], kint["vti"][:, cs], W["wv2"][:],
                                 start=False, stop=True)
                vt = vpp.tile([128, 258], BF16, name=f"vp{hs}_{ch}",
                              tag=f"vp{hs}_{ch}")
                vp[(hs, ch)] = vt
                if ch % 2 == 0:
                    nc.vector.tensor_copy(vt[:, 0:256], ps[:, 0:256])
                else:
                    nc.scalar.copy(vt[:, 0:256], ps[:, 0:256])
                nc.vector.memset(vt[:, 256:258], 1.0)

        kv_prep(0)

        # ---- gate projection (scalar copies: before any sqrt/exp) ----
        gT = {}
        for part in ("r", "i"):
            t = qpp.tile([128, 1536], F32, name=f"gT{part}", tag=f"gT{part}")
            gT[part] = t
            for ch in range(3):
                cs = slice(ch * 512, (ch + 1) * 512)
                ps = scp.tile([128, 1024], F32, name="scps", tag="sc")
                if part == "r":
                    nc.tensor.matmul(ps[:, 0:512], W["wg_r"][:], qtr[:, cs],
                                     start=True, stop=False)
                    nc.tensor.matmul(ps[:, 0:512], W["wg_in"][:], qti[:, cs],
                                     start=False, stop=True)
                else:
                    nc.tensor.matmul(ps[:, 0:512], W["wg_i"][:], qtr[:, cs],
                                     start=True, stop=False)
                    nc.tensor.matmul(ps[:, 0:512], W["wg_r"][:], qti[:, cs],
                                     start=False, stop=True)
                nc.scalar.copy(t[:, cs], ps[:, 0:512])

        qinctx.close()   # qtr/qti/pqr/pqi dead; free 24KB before hot tags

        # =========== pipeline ===========
        # branch-step sequence: (b, br) for b in 0..2, br in 0,1
        steps = [(b, br) for b in range(NB) for br in (0, 1)]
        score_ps = {}   # (b, br) -> list of 8 psum tiles
        et = {}         # (b, br, pair) -> [128,1024] bf16
        avt = {}        # (b, br) -> list of 4 av psum tiles [128,258]
        av_sb = {}      # (b, uu) -> br0 av drained [128,257]
        EP = {}         # per-batch epilogue intermediates

        def emit_scores(b, br):
            hs = 0 if b < 2 else 1
            qs = slice(b * 512, (b + 1) * 512)
            tiles = []
            for ch in range(8):
                cs = slice(ch * 128, (ch + 1) * 128)
                ps = scp.tile([128, 1024], F32, name="scps", tag="sc")
                nc.tensor.matmul(ps[:, 0:512], kp[(hs, "r")][:, cs],
                                 qp[(br, "r")][:, qs], start=True, stop=False)
                nc.tensor.matmul(ps[:, 0:512], kp[(hs, "i")][:, cs],
                                 qp[(br, "i")][:, qs], start=False, stop=True)
                nc.tensor.matmul(ps[:, 512:1024], kp[(hs, "r")][:, cs],
                                 qp[(br, "i")][:, qs], start=True, stop=False)
                nc.tensor.matmul(ps[:, 512:1024], kp[(hs, "in")][:, cs],
                                 qp[(br, "r")][:, qs], start=False, stop=True)
                tiles.append(ps)
            score_ps[(b, br)] = tiles

        # square modes per chunk: only ONE PSUM input allowed per op.
        # 'S': Scalar Square drains PSUM directly (cheapest single op)
        # 'D': DVE copy PSUM->SBUF then DVE multiply
        # 'G': DVE copy PSUM->SBUF then GpSimd multiply
        SQMODE = ['S', 'D', 'G', 'D', 'S', 'D', 'G', 'D']
        S2Q = {}

        def emit_sq_s2(b, br):
            # two [128,2048] quads per branch; adds split DVE/GpSimd
            s2_tiles = []
            for p in range(2):
                s2 = hot.tile([128, 2048], F32, name="s2", tag="s2", bufs=3)
                s2_tiles.append(s2)
            for ch in range(8):
                ps = score_ps[(b, br)][ch]
                sq = hot.tile([128, 1024], F32, name="sq", tag="sq", bufs=2)
                mode = SQMODE[ch]
                if mode == 'S':
                    nc.scalar.activation(sq[:], ps[:], AF.Square)
                else:
                    cp = hot.tile([128, 1024], F32, name="cp", tag="cp",
                                  bufs=2)
                    nc.vector.tensor_copy(cp[:], ps[:])
                    if mode == 'D':
                        nc.vector.tensor_mul(sq[:], cp[:], cp[:])
                    else:
                        nc.gpsimd.tensor_mul(sq[:], cp[:], cp[:])
                s2 = s2_tiles[ch // 4]
                slot = slice((ch % 4) * 512, (ch % 4 + 1) * 512)
                if ch % 8 in (0, 3, 5):
                    nc.vector.scalar_tensor_tensor(
                        s2[:, slot], sq[:, 0:512], 1.0, sq[:, 512:1024],
                        op0=OP.mult, op1=OP.add)
                else:
                    nc.gpsimd.tensor_add(s2[:, slot], sq[:, 0:512],
                                         sq[:, 512:1024])
            score_ps.pop((b, br))
            S2Q[(b, br)] = s2_tiles
            return s2_tiles

        def emit_sqrt(b, br, s2_tiles, fine=False):
            # in-place quad sqrt (mag overwrites s2); fine= pair granularity
            if fine:
                for p in range(2):
                    for h in (slice(0, 1024), slice(1024, 2048)):
                        nc.scalar.activation(s2_tiles[p][:, h],
                                             s2_tiles[p][:, h], AF.Sqrt,
                                             bias=eps8[:])
            else:
                for p in range(2):
                    nc.scalar.activation(s2_tiles[p][:], s2_tiles[p][:],
                                         AF.Sqrt, bias=eps8[:])
            return s2_tiles

        def emit_exp(b, br, mags, fine=False):
            # fake dep: exp-q0's bias is produced from mag-q1, forcing both
            # sqrts to run before any exp (keeps table switches at 2/branch);
            # fine= latency-optimized half ops for the pipeline tail
            if fine:
                for p in range(2):
                    e = hot.tile([128, 2048], BF16, name="et", tag="et",
                                 bufs=3)
                    for h in (slice(0, 1024), slice(1024, 2048)):
                        nc.scalar.activation(e[:, h], mags[p][:, h], AF.Exp,
                                             scale=SCALE)
                    et[(b, br, p)] = e
                return
            z = epi.tile([128, 1], F32, name="zb", tag="zb")
            nc.vector.tensor_scalar_mul(z[:], mags[1][:, 0:1], 0.0)
            for p in range(2):
                e = hot.tile([128, 2048], BF16, name="et", tag="et", bufs=3)
                nc.scalar.activation(e[:], mags[p][:], AF.Exp, scale=SCALE,
                                     bias=z[:] if p == 0 else 0.0)
                et[(b, br, p)] = e

        def emit_av(b, br):
            hs = 0 if b < 2 else 1
            avs = []
            for uu in range(4):
                av = avp.tile([128, 258], F32, name=f"av{uu}", tag="av")
                avs.append(av)
            for c in range(8):
                e = et[(b, br, c // 4)]
                base = (c % 4) * 512
                for uu in range(4):
                    nc.tensor.matmul(
                        avs[uu][:],
                        e[:, base + uu * 128: base + (uu + 1) * 128],
                        vp[(hs, c)][:],
                        start=(c == 0), stop=(c == 7))
            avt[(b, br)] = avs

        def emit_drain0(b):
            # after av(b,0): gather rowsums, recip, normalize-drain to SBUF
            d = EP.setdefault(b, {})
            s1 = epi.tile([128, 4], F32, name="s1g", tag="s1g")
            for uu in range(4):
                nc.vector.tensor_copy(s1[:, uu:uu + 1],
                                      avt[(b, 0)][uu][:, 256:257])
            inv1 = epi.tile([128, 4], F32, name="inv1", tag="inv1")
            nc.vector.reciprocal(inv1[:], s1[:])
            anps = []
            for uu in range(4):
                anp = epi.tile([128, 512], F32, name="anp", tag="anp", bufs=4)
                nc.vector.tensor_scalar_mul(anp[:, 0:256],
                                            avt[(b, 0)][uu][:, 0:256],
                                            inv1[:, uu:uu + 1])
                anps.append(anp)
            d["anp"] = anps

        def emit_drain1(b):
            # after av(b,1): same for branch 1, then rms accum
            d = EP[b]
            s2g = epi.tile([128, 4], F32, name="s2g", tag="s2g")
            for uu in range(4):
                nc.vector.tensor_copy(s2g[:, uu:uu + 1],
                                      avt[(b, 1)][uu][:, 256:257])
            inv2 = epi.tile([128, 4], F32, name="inv2", tag="inv2")
            nc.vector.reciprocal(inv2[:], s2g[:])
            ss = epi.tile([128, 4], F32, name="ss", tag="ss")
            scr = epi.tile([128, 512], F32, name="scr", tag="scr", bufs=1)
            for uu in range(4):
                anp = d["anp"][uu]
                nc.vector.tensor_scalar_mul(anp[:, 256:512],
                                            avt[(b, 1)][uu][:, 0:256],
                                            inv2[:, uu:uu + 1])
                nc.vector.scalar_tensor_tensor(
                    scr[:], anp[:], 1.0, anp[:], op0=OP.mult, op1=OP.mult,
                    accum_out=ss[:, uu:uu + 1])
            d["ss"] = ss

        def emit_ep_rms(b):
            # Scalar sqrt [128,4]; fake-dep bias ties it into the next
            # batch's sqrt-table block to avoid extra table loads
            d = EP[b]
            rms = epi.tile([128, 4], F32, name="rms", tag="rms")
            nxt = S2Q.get((b + 1, 0))
            if nxt is not None:
                z5 = epi.tile([128, 1], F32, name="z5", tag="z5")
                nc.vector.tensor_scalar(z5[:], nxt[0][:, 0:1], 0.0, 1e-5,
                                        op0=OP.mult, op1=OP.add)
                bias = z5[:]
            else:
                bias = eps5[:]
            nc.scalar.activation(rms[:], d["ss"][:], AF.Sqrt, bias=bias,
                                 scale=1.0 / 256.0)
            d["rms"] = rms

        def emit_ep_rinv(b):
            d = EP[b]
            rinv = epi.tile([128, 4], F32, name="rinv", tag="rinv")
            nc.vector.reciprocal(rinv[:], d["rms"][:])
            d["rinv"] = rinv

        def emit_ep_t(b):
            # GpSimd (SBUF only): t_u = a1n + (-lam)*a2n; rinv folded into
            # the final output copy instead
            d = EP[b]
            ts = []
            for uu in range(4):
                anp = d["anp"][uu]
                t = epi.tile([128, 256], F32, name="tt", tag="tt", bufs=4)
                nc.vector.scalar_tensor_tensor(
                    t[:], anp[:, 256:512], lam_t[:], anp[:, 0:256],
                    op0=OP.mult, op1=OP.add)
                ts.append(t)
            d["t"] = ts
        def emit_ep_transp(b):
            # PE: per unit-pair transpose into one scp tile
            d = EP[b]
            tp = scp.tile([128, 1024], F32, name="scps", tag="sc")
            for pi in range(2):
                for j in range(2):       # unit within pair
                    u = 2 * pi + j
                    base = pi * 512
                    nc.tensor.transpose(tp[:, base + j * 128: base + (j + 1) * 128],
                                        d["t"][u][:, 0:128], ident[:])
                    nc.tensor.transpose(tp[:, base + 256 + j * 128: base + 256 + (j + 1) * 128],
                                        d["t"][u][:, 128:256], ident[:])
            d["tp"] = tp

        def emit_ep_gating(b):
            d = EP[b]
            tp = d["tp"]
            crs = []
            for pi in range(2):
                gcols = slice((b * 4 + 2 * pi) * 128, (b * 4 + 2 * pi + 2) * 128)
                base = pi * 512
                m1 = epi.tile([128, 256], F32, name="m1", tag="m1", bufs=1)
                m2 = epi.tile([128, 256], F32, name="m2", tag="m2", bufs=1)
                m3 = epi.tile([128, 256], F32, name="m3", tag="m3", bufs=1)
                m4 = epi.tile([128, 256], F32, name="m4", tag="m4", bufs=1)
                nc.vector.tensor_mul(m1[:], gT["r"][:, gcols], tp[:, base:base + 256])
                nc.vector.tensor_mul(m2[:], gT["i"][:, gcols], tp[:, base + 256:base + 512])
                nc.vector.tensor_mul(m3[:], gT["i"][:, gcols], tp[:, base:base + 256])
                nc.vector.tensor_mul(m4[:], gT["r"][:, gcols], tp[:, base + 256:base + 512])
                cr = epi.tile([128, 256], F32R, name="cr", tag="cr")
                ci = epi.tile([128, 256], F32R, name="ci", tag="ci")
                nc.gpsimd.tensor_sub(cr[:], m1[:], m2[:])
                nc.gpsimd.tensor_add(ci[:], m3[:], m4[:])
                crs.append((cr, ci))
            d["cr"] = crs
        def emit_ep_outproj(b):
            d = EP[b]
            po = scp.tile([128, 1024], F32, name="scps", tag="sc")
            for uu in range(4):
                cr, ci = d["cr"][uu // 2]
                j = uu % 2
                pos = slice(uu * 256, (uu + 1) * 256)
                nc.tensor.matmul(po[:, pos], cr[:, j * 128:(j + 1) * 128],
                                 W["wo1"][:], start=True, stop=False)
                nc.tensor.matmul(po[:, pos], ci[:, j * 128:(j + 1) * 128],
                                 W["wo2"][:], start=False, stop=True)
            d["po"] = po

        def emit_ep_out(b):
            # Scalar copies PSUM->SBUF, sync DMAs out
            d = EP[b]
            for uu in range(4):
                iu = b * 4 + uu
                ot = osb.tile([128, 256], F32, name="ot", tag="ot")
                nc.scalar.mul(ot[:], d["po"][:, uu * 256:(uu + 1) * 256],
                              d["rinv"][:, uu:uu + 1])
                nc.sync.dma_start(out_d[iu * 128:(iu + 1) * 128, :], ot[:])

        # ---- emission sequence ----
        emit_scores(0, 0)
        s2_00 = emit_sq_s2(0, 0)
        mag_00 = emit_sqrt(0, 0, s2_00)
        emit_exp(0, 0, mag_00)

        emit_scores(0, 1)
        s2_01 = emit_sq_s2(0, 1)
        mag_01 = emit_sqrt(0, 1, s2_01)
        emit_exp(0, 1, mag_01)
        emit_av(0, 0)
        emit_drain0(0)

        # prep head slot 1 here: its v-proj PSUM allocations rotate through
        # the av pool while sc(1,0) runs, instead of stalling batch 1->2
        load_k_side(1, nc.sync)
        kv_prep(1)
        kinctx.close()

        emit_scores(1, 0)
        s2_10 = emit_sq_s2(1, 0)
        mag_10 = emit_sqrt(1, 0, s2_10)
        emit_exp(1, 0, mag_10)
        emit_av(0, 1)

        emit_drain1(0)
        emit_ep_rms(0)
        emit_ep_rinv(0)
        emit_ep_t(0)
        emit_ep_transp(0)
        emit_ep_gating(0)
        emit_ep_outproj(0)

        emit_scores(1, 1)
        s2_11 = emit_sq_s2(1, 1)
        mag_11 = emit_sqrt(1, 1, s2_11)
        emit_ep_out(0)
        emit_exp(1, 1, mag_11)
        emit_av(1, 0)
        emit_drain0(1)

        emit_scores(2, 0)
        s2_20 = emit_sq_s2(2, 0)
        mag_20 = emit_sqrt(2, 0, s2_20)
        emit_exp(2, 0, mag_20)
        emit_av(1, 1)

        emit_drain1(1)
        emit_ep_rms(1)
        emit_ep_rinv(1)
        emit_ep_t(1)
        emit_ep_transp(1)
        emit_ep_gating(1)
        emit_ep_outproj(1)

        emit_scores(2, 1)
        s2_21 = emit_sq_s2(2, 1)
        mag_21 = emit_sqrt(2, 1, s2_21, fine=True)
        emit_ep_out(1)
        emit_exp(2, 1, mag_21, fine=True)
        emit_av(2, 0)
        emit_drain0(2)
        emit_av(2, 1)

        emit_drain1(2)
        emit_ep_rms(2)
        emit_ep_rinv(2)
        emit_ep_t(2)
        emit_ep_transp(2)
        emit_ep_gating(2)
        emit_ep_outproj(2)
        emit_ep_out(2)

    nc.compile()
    return nc


def _get_program():
    if "nc" not in _prog_cache:
        _prog_cache["nc"] = _build_program()
    return _prog_cache["nc"]


def _prep_inputs(inputs):
    f = {k: np.asarray(v, dtype=np.float32) for k, v in inputs.items()}
    lam1 = np.float32(np.exp(np.float32(np.sum(f["lq1"] * f["lk1"]))))
    lam2 = np.float32(np.exp(np.float32(np.sum(f["lq2"] * f["lk2"]))))
    x = np.float32(lam1 - lam2 + np.float32(LAMBDA_INIT))
    lam = np.float32(1.0 / (1.0 + np.exp(-x)))

    wq_rT = f["qw_r"].T.copy()          # [128, 256]
    wq_iT = f["qw_i"].T.copy()
    wk_rT = f["kw_r"].T.copy()          # [128, 128]
    wk_iT = f["kw_i"].T.copy()
    vw_rT = f["vw_r"].T; vw_iT = f["vw_i"].T
    wv1 = np.concatenate([vw_rT, vw_iT], 1).copy()
    wv2 = np.concatenate([-vw_iT, vw_rT], 1).copy()
    wg_rT = f["gw_r"].T.copy(); wg_iT = f["gw_i"].T.copy()
    ow_rT = f["ow_r"].T; ow_iT = f["ow_i"].T
    wo1 = np.concatenate([ow_rT, ow_iT], 1).copy()
    wo2 = np.concatenate([-ow_iT, ow_rT], 1).copy()
    shared = {
        "wq_r": wq_rT, "wq_i": wq_iT, "wq_in": (-wq_iT).copy(),
        "wk_r": wk_rT, "wk_i": wk_iT, "wk_in": (-wk_iT).copy(),
        "wv1": wv1, "wv2": wv2,
        "wg_r": wg_rT, "wg_i": wg_iT, "wg_in": (-wg_iT).copy(),
        "wo1": wo1, "wo2": wo2,
        "lamneg": np.full((128, 1), -lam, np.float32),
    }

    in_maps = []
    for c in range(NCORES):
        units = _core_units(c)
        heads = [units[0][0], units[8][0]]
        m = dict(shared)

        def pack_q(t):
            cols = [t[0, h, q * 128:(q + 1) * 128, :].T for (h, q) in units]
            return np.ascontiguousarray(np.concatenate(cols, 1))
        m["qT_r"] = pack_q(f["q_r"]); m["qT_i"] = pack_q(f["q_i"])
        m["peq_r"] = pack_q(f["pe_q_r"]); m["peq_i"] = pack_q(f["pe_q_i"])

        def pack_k(t):
            return np.ascontiguousarray(
                np.concatenate([t[0, h].T for h in heads], 1))
        m["kT_r"] = pack_k(f["k_r"]); m["kT_i"] = pack_k(f["k_i"])
        m["pek_r"] = pack_k(f["pe_k_r"]); m["pek_i"] = pack_k(f["pe_k_i"])
        m["vT_r"] = pack_k(f["v_r"]); m["vT_i"] = pack_k(f["v_i"])
        in_maps.append(m)
    return in_maps


def _unpack(results):
    out_r = np.zeros((1, H, S, D), np.float32)
    out_i = np.zeros((1, H, S, D), np.float32)
    for c in range(NCORES):
        o = results[c]["out"]
        for u, (h, q) in enumerate(_core_units(c)):
            blk = o[u * 128:(u + 1) * 128]
            out_r[0, h, q * 128:(q + 1) * 128, :] = blk[:, 0:128]
            out_i[0, h, q * 128:(q + 1) * 128, :] = blk[:, 128:256]
    return out_r, out_i


def _run(inputs, trace=False, tmpdir=None):
    nc = _get_program()
    in_maps = _prep_inputs(inputs)
    res = run_bass_kernel_spmd(nc, in_maps, list(range(NCORES)), trace=trace,
                               tmpdir=tmpdir)
    return _unpack(res.results), res


def kernel(**inputs):
    (out_r, out_i), _ = _run(inputs, trace=False)
    return out_r, out_i


# revision 19
# speedup vs baseline: 1.0762x; 1.0762x over previous
"""Trainium2 Bass kernel for ComplexDifferentialAttention.

Sharding: 96 (head, q-tile-of-128) units over 8 cores; each core gets
8 q-tiles of one head (A) + 4 q-tiles of another head (B), so the SPMD
program is identical on every core: 3 batches of 4 q-tiles with
head-slot pattern (A, A, B).

Engine plan (v2):
- PE stream: sc(0,0) | sc(0,1) av(0,0) | sc(1,0) av(0,1) ep(0) |
  sc(1,1) av(1,0) | sc(2,0) av(1,1) ep(1) | sc(2,1) av(2,0) | av(2,1)
  ep(2) -- kept dense so the PE pstate ramps to full clock.
- DVE: squares of score PSUM (drains banks), epilogue accums/scalars.
- GpSimd: half-adds (|s|^2), av drains, u/t builds, half the gating.
- Scalar: ONLY sqrt and exp, in per-branch blocks (sqrt x4-pairs then
  exp x4-pairs) so activation-table reloads drop to ~4/batch; rms and
  output copies slotted into matching table groups.
- Softmax row-sum (ones-column trick) and RMS normalization folded into
  per-query scalars c1 = rinv/sum1, c2 = -lam*rinv/sum2, batched
  [128,4]-wide across the 4 units of a batch.
- et and vp in bf16 (prob/value matmul), everything pre-exp in f32.
"""
import sys, os, math
sys.path.insert(0, '/opt/trn_rl_repo')
import numpy as np
from contextlib import ExitStack

import concourse.bacc as bacc
import concourse.tile as tile
from concourse import mybir
from concourse.bass_utils import run_bass_kernel_spmd
from concourse.masks import make_identity

F32 = mybir.dt.float32
F32R = mybir.dt.float32r
BF16 = mybir.dt.bfloat16
F16 = mybir.dt.float16
AF = mybir.ActivationFunctionType
OP = mybir.AluOpType

D = 128
S = 1024
H = 12
NCORES = 8
NB = 3          # batches per core, 4 units each
SCALE = 1.0 / math.sqrt(D)
LAMBDA_INIT = 0.8 - 0.6 * math.exp(-0.3)

_prog_cache = {}


def _core_units(c):
    """Units for core c: list of (head, qtile). 8 of head A + 4 of head B."""
    k, odd = divmod(c, 2)
    hA = 3 * k + odd          # cores 2k -> 3k ; 2k+1 -> 3k+1
    hB = 3 * k + 2
    qoff = 0 if odd == 0 else 4
    return [(hA, q) for q in range(8)] + [(hB, qoff + q) for q in range(4)]


def _build_program():
    nc = bacc.Bacc("TRN2", target_bir_lowering=False, debug=False,
                   num_devices=NCORES)

    def din(name, shape, dt=F32R):
        return nc.dram_tensor(name, shape, dt, kind="ExternalInput").ap()

    qT_r = din("qT_r", [128, 12 * 128]); qT_i = din("qT_i", [128, 12 * 128])
    peq_r = din("peq_r", [128, 12 * 128], F32)
    peq_i = din("peq_i", [128, 12 * 128], F32)
    kT_r = din("kT_r", [128, 2048]); kT_i = din("kT_i", [128, 2048])
    pek_r = din("pek_r", [128, 2048], F32); pek_i = din("pek_i", [128, 2048], F32)
    vT_r = din("vT_r", [128, 2048]); vT_i = din("vT_i", [128, 2048])
    wq_r = din("wq_r", [128, 256]); wq_i = din("wq_i", [128, 256])
    wq_in = din("wq_in", [128, 256])
    wk_r = din("wk_r", [128, 128]); wk_i = din("wk_i", [128, 128])
    wk_in = din("wk_in", [128, 128])
    wv1 = din("wv1", [128, 256]); wv2 = din("wv2", [128, 256])
    wg_r = din("wg_r", [128, 128]); wg_i = din("wg_i", [128, 128])
    wg_in = din("wg_in", [128, 128])
    wo1 = din("wo1", [128, 256]); wo2 = din("wo2", [128, 256])
    lamneg = din("lamneg", [128, 1], F32)
    out_d = nc.dram_tensor("out", [12 * 128, 256], F32, kind="ExternalOutput").ap()

    with tile.TileContext(nc) as tc, ExitStack() as ctx:
        cst = ctx.enter_context(tc.tile_pool(name="cst", bufs=1))
        qpp = ctx.enter_context(tc.tile_pool(name="qpp", bufs=1))
        kpp = ctx.enter_context(tc.tile_pool(name="kpp", bufs=1))
        vpp = ctx.enter_context(tc.tile_pool(name="vpp", bufs=1))
        hot = ctx.enter_context(tc.tile_pool(name="hot", bufs=1))
        epi = ctx.enter_context(tc.tile_pool(name="epi", bufs=2))
        osb = ctx.enter_context(tc.tile_pool(name="osb", bufs=2))
        scp = ctx.enter_context(tc.tile_pool(name="scp", bufs=2, space="PSUM"))
        avp = ctx.enter_context(tc.tile_pool(name="avp", bufs=4, space="PSUM"))
        # input pools closed right after their last use to free SBUF
        # (LIFO: qin sits on top of kin so it can close first)
        kinctx = ExitStack()
        kin = kinctx.enter_context(tc.tile_pool(name="kin", bufs=1))
        qinctx = ExitStack()
        qin = qinctx.enter_context(tc.tile_pool(name="qin", bufs=1))

        # ---- weights + constants (gpsimd DMA queue: ~25ns issue) ----
        W = {}
        for nm, ap, w in [("wq_r", wq_r, 256), ("wq_i", wq_i, 256),
                          ("wq_in", wq_in, 256), ("wk_r", wk_r, 128),
                          ("wk_i", wk_i, 128), ("wk_in", wk_in, 128),
                          ("wv1", wv1, 256), ("wv2", wv2, 256),
                          ("wg_r", wg_r, 128), ("wg_i", wg_i, 128),
                          ("wg_in", wg_in, 128), ("wo1", wo1, 256),
                          ("wo2", wo2, 256)]:
            t = cst.tile([128, w], F32R, name=f"w_{nm}", tag=f"w_{nm}")
            q = nc.sync if nm.startswith("wq") else nc.scalar
            q.dma_start(t[:], ap[:])
            W[nm] = t
        lam_t = cst.tile([128, 1], F32)
        nc.scalar.dma_start(lam_t[:], lamneg[:])
        ident = cst.tile([128, 128], F32)
        make_identity(nc, ident[:])
        eps8 = cst.tile([128, 1], F32)
        nc.vector.memset(eps8[:], 1e-8)
        eps5 = cst.tile([128, 1], F32)
        nc.vector.memset(eps5[:], 1e-5)

        # ---- q-side inputs (chunked DMAs so matmuls start early) ----
        qtr = qin.tile([128, 1536], F32R, name="qtr", tag="qtr")
        qti = qin.tile([128, 1536], F32R, name="qti", tag="qti")
        pqr = qin.tile([128, 1536], F32, name="pqr", tag="pqr")
        pqi = qin.tile([128, 1536], F32, name="pqi", tag="pqi")
        for ch in range(3):
            cs = slice(ch * 512, (ch + 1) * 512)
            nc.sync.dma_start(qtr[:, cs], qT_r[:, cs])
            nc.sync.dma_start(qti[:, cs], qT_i[:, cs])
            nc.scalar.dma_start(pqr[:, cs], peq_r[:, cs])
            nc.scalar.dma_start(pqi[:, cs], peq_i[:, cs])

        kint = {}
        for nm in ("ktr", "kti", "pkr", "pki", "vtr", "vti"):
            kint[nm] = kin.tile([128, 1024], F32 if nm in ("pkr", "pki")
                                else F32R, name=nm, tag=nm)

        def load_k_side(hs, q):
            ks_ = slice(hs * 1024, (hs + 1) * 1024)
            for nm, src in [("ktr", kT_r), ("kti", kT_i), ("vtr", vT_r),
                            ("vti", vT_i), ("pkr", pek_r), ("pki", pek_i)]:
                for ch in range(2):
                    cs = slice(ch * 512, (ch + 1) * 512)
                    gs = slice(hs * 1024 + ch * 512, hs * 1024 + (ch + 1) * 512)
                    q.dma_start(kint[nm][:, cs], src[:, gs])

        load_k_side(0, nc.gpsimd)

        # ---- q projection (+pe_q) ----
        qp = {}
        for half in (0, 1):
            hs_ = slice(half * 128, (half + 1) * 128)
            for part in ("r", "i"):
                t = qpp.tile([128, 1536], F32R, name=f"qp{half}{part}",
                             tag=f"qp{half}{part}")
                qp[(half, part)] = t
                for ch in range(3):
                    cs = slice(ch * 512, (ch + 1) * 512)
                    ps = scp.tile([128, 1024], F32, name="scps", tag="sc")
                    if part == "r":
                        nc.tensor.matmul(ps[:, 0:512], W["wq_r"][:, hs_],
                                         qtr[:, cs], start=True, stop=False)
                        nc.tensor.matmul(ps[:, 0:512], W["wq_in"][:, hs_],
                                         qti[:, cs], start=False, stop=True)
                        pe = pqr
                    else:
                        nc.tensor.matmul(ps[:, 0:512], W["wq_i"][:, hs_],
                                         qtr[:, cs], start=True, stop=False)
                        nc.tensor.matmul(ps[:, 0:512], W["wq_r"][:, hs_],
                                         qti[:, cs], start=False, stop=True)
                        pe = pqi
                    nc.vector.tensor_add(t[:, cs], ps[:, 0:512], pe[:, cs])

        # ---- K/V prep for a head slot ----
        kp = {}   # (hs, 'r'|'i'|'in') -> [o=128, k=1024] f32r
        vp = {}   # (hs, chunk) -> [k=128, 258] bf16  ([vp_r | vp_i | 1])

        def kv_prep(hs):
            for part in ("r", "i"):
                t = kpp.tile([128, 1024], F32R, name=f"kp{hs}{part}",
                             tag=f"kp{hs}{part}")
                kp[(hs, part)] = t
                for ch in range(2):
                    cs = slice(ch * 512, (ch + 1) * 512)
                    ps = scp.tile([128, 1024], F32, name="scps", tag="sc")
                    if part == "r":
                        nc.tensor.matmul(ps[:, 0:512], W["wk_r"][:],
                                         kint["ktr"][:, cs], start=True, stop=False)
                        nc.tensor.matmul(ps[:, 0:512], W["wk_in"][:],
                                         kint["kti"][:, cs], start=False, stop=True)
                        pe = kint["pkr"]
                    else:
                        nc.tensor.matmul(ps[:, 0:512], W["wk_i"][:],
                                         kint["ktr"][:, cs], start=True, stop=False)
                        nc.tensor.matmul(ps[:, 0:512], W["wk_r"][:],
                                         kint["kti"][:, cs], start=False, stop=True)
                        pe = kint["pki"]
                    nc.vector.tensor_add(t[:, cs], ps[:, 0:512], pe[:, cs])
            tn = kpp.tile([128, 1024], F32R, name=f"kp{hs}in", tag=f"kp{hs}in")
            kp[(hs, "in")] = tn
            nc.vector.tensor_scalar_mul(tn[:], kp[(hs, "i")][:], -1.0)
            for ch in range(8):
                cs = slice(ch * 128, (ch + 1) * 128)
                ps = avp.tile([128, 258], F32, name="vps", tag="av")
                nc.tensor.matmul(ps[:, 0:256], kint["vtr"][:, cs], W["wv1"][:],
                                 start=True, stop=False)
                nc.tensor.matmul(ps[:, 0:256], kint["vti"][:, cs], W["wv2"][:],
                                 start=False, stop=True)
                vt = vpp.tile([128, 258], BF16, name=f"vp{hs}_{ch}",
                              tag=f"vp{hs}_{ch}")
                vp[(hs, ch)] = vt
                if ch % 2 == 0:
                    nc.vector.tensor_copy(vt[:, 0:256], ps[:, 0:256])
                else:
                    nc.scalar.copy(vt[:, 0:256], ps[:, 0:256])
                nc.vector.memset(vt[:, 256:258], 1.0)

        kv_prep(0)

        # ---- gate projection (scalar copies: before any sqrt/exp) ----
        gT = {}
        for part in ("r", "i"):
            t = qpp.tile([128, 1536], F32, name=f"gT{part}", tag=f"gT{part}")
            gT[part] = t
            for ch in range(3):
                cs = slice(ch * 512, (ch + 1) * 512)
                ps = scp.tile([128, 1024], F32, name="scps", tag="sc")
                if part == "r":
                    nc.tensor.matmul(ps[:, 0:512], W["wg_r"][:], qtr[:, cs],
                                     start=True, stop=False)
                    nc.tensor.matmul(ps[:, 0:512], W["wg_in"][:], qti[:, cs],
                                     start=False, stop=True)
                else:
                    nc.tensor.matmul(ps[:, 0:512], W["wg_i"][:], qtr[:, cs],
                                     start=True, stop=False)
                    nc.tensor.matmul(ps[:, 0:512], W["wg_r"][:], qti[:, cs],
                                     start=False, stop=True)
                nc.scalar.copy(t[:, cs], ps[:, 0:512])

        qinctx.close()   # qtr/qti/pqr/pqi dead; free 24KB before hot tags

        # =========== pipeline ===========
        # branch-step sequence: (b, br) for b in 0..2, br in 0,1
        steps = [(b, br) for b in range(NB) for br in (0, 1)]
        score_ps = {}   # (b, br) -> list of 8 psum tiles
        et = {}         # (b, br, pair) -> [128,1024] bf16
        avt = {}        # (b, br) -> list of 4 av psum tiles [128,258]
        av_sb = {}      # (b, uu) -> br0 av drained [128,257]
        EP = {}         # per-batch epilogue intermediates

        def emit_scores(b, br):
            hs = 0 if b < 2 else 1
            qs = slice(b * 512, (b + 1) * 512)
            tiles = []
            for ch in range(8):
                cs = slice(ch * 128, (ch + 1) * 128)
                ps = scp.tile([128, 1024], F32, name="scps", tag="sc")
                nc.tensor.matmul(ps[:, 0:512], kp[(hs, "r")][:, cs],
                                 qp[(br, "r")][:, qs], start=True, stop=False)
                nc.tensor.matmul(ps[:, 0:512], kp[(hs, "i")][:, cs],
                                 qp[(br, "i")][:, qs], start=False, stop=True)
                nc.tensor.matmul(ps[:, 512:1024], kp[(hs, "r")][:, cs],
                                 qp[(br, "i")][:, qs], start=True, stop=False)
                nc.tensor.matmul(ps[:, 512:1024], kp[(hs, "in")][:, cs],
                                 qp[(br, "r")][:, qs], start=False, stop=True)
                tiles.append(ps)
            score_ps[(b, br)] = tiles

        # square modes per chunk: only ONE PSUM input allowed per op.
        # 'S': Scalar Square drains PSUM directly (cheapest single op)
        # 'D': DVE copy PSUM->SBUF then DVE multiply
        # 'G': DVE copy PSUM->SBUF then GpSimd multiply
        SQMODE = ['S', 'D', 'D', 'D', 'S', 'D', 'G', 'D']
        S2Q = {}

        def emit_sq_s2(b, br):
            # fp16 square/add path (2x DVE rate); x0.5 prescale keeps
            # sq = (x/2)^2 within fp16 range; exp scale doubled later
            s2_tiles = []
            for p in range(2):
                s2 = hot.tile([128, 2048], F16, name="s2", tag="s2", bufs=3)
                s2_tiles.append(s2)
            for ch in range(8):
                ps = score_ps[(b, br)][ch]
                sq = hot.tile([128, 1024], F16, name="sq", tag="sq", bufs=3)
                mode = SQMODE[ch]
                if mode == 'S':
                    nc.scalar.activation(sq[:], ps[:], AF.Square, scale=0.5)
                else:
                    cp = hot.tile([128, 1024], F16, name="cp", tag="cp",
                                  bufs=3)
                    nc.vector.tensor_scalar_mul(cp[:], ps[:], 0.5)
                    if mode == 'D':
                        nc.vector.tensor_mul(sq[:], cp[:], cp[:])
                    else:
                        nc.gpsimd.tensor_mul(sq[:], cp[:], cp[:])
                s2 = s2_tiles[ch // 4]
                slot = slice((ch % 4) * 512, (ch % 4 + 1) * 512)
                if ch % 8 in (0, 3):
                    nc.vector.scalar_tensor_tensor(
                        s2[:, slot], sq[:, 0:512], 1.0, sq[:, 512:1024],
                        op0=OP.mult, op1=OP.add)
                else:
                    nc.gpsimd.tensor_add(s2[:, slot], sq[:, 0:512],
                                         sq[:, 512:1024])
            score_ps.pop((b, br))
            S2Q[(b, br)] = s2_tiles
            return s2_tiles

        def emit_sqrt(b, br, s2_tiles, fine=False):
            # in-place quad sqrt (mag overwrites s2); fine= pair granularity
            if fine:
                for p in range(2):
                    for h in (slice(0, 1024), slice(1024, 2048)):
                        nc.scalar.activation(s2_tiles[p][:, h],
                                             s2_tiles[p][:, h], AF.Sqrt,
                                             bias=eps8[:])
            else:
                for p in range(2):
                    nc.scalar.activation(s2_tiles[p][:], s2_tiles[p][:],
                                         AF.Sqrt, bias=eps8[:])
            return s2_tiles

        def emit_exp(b, br, mags, fine=False):
            # fake dep: exp-q0's bias is produced from mag-q1, forcing both
            # sqrts to run before any exp (keeps table switches at 2/branch);
            # fine= latency-optimized half ops for the pipeline tail
            if fine:
                for p in range(2):
                    e = hot.tile([128, 2048], BF16, name="et", tag="et",
                                 bufs=3)
                    for h in (slice(0, 1024), slice(1024, 2048)):
                        nc.scalar.activation(e[:, h], mags[p][:, h], AF.Exp,
                                             scale=2.0 * SCALE)
                    et[(b, br, p)] = e
                return
            z = epi.tile([128, 1], F32, name="zb", tag="zb")
            nc.vector.tensor_scalar_mul(z[:], mags[1][:, 0:1], 0.0)
            for p in range(2):
                e = hot.tile([128, 2048], BF16, name="et", tag="et", bufs=3)
                nc.scalar.activation(e[:], mags[p][:], AF.Exp,
                                     scale=2.0 * SCALE,
                                     bias=z[:] if p == 0 else 0.0)
                et[(b, br, p)] = e

        def emit_av(b, br):
            hs = 0 if b < 2 else 1
            avs = []
            for uu in range(4):
                av = avp.tile([128, 258], F32, name=f"av{uu}", tag="av")
                avs.append(av)
            for c in range(8):
                e = et[(b, br, c // 4)]
                base = (c % 4) * 512
                for uu in range(4):
                    nc.tensor.matmul(
                        avs[uu][:],
                        e[:, base + uu * 128: base + (uu + 1) * 128],
                        vp[(hs, c)][:],
                        start=(c == 0), stop=(c == 7))
            avt[(b, br)] = avs

        def emit_drain0(b):
            # after av(b,0): gather rowsums, recip, normalize-drain to SBUF
            d = EP.setdefault(b, {})
            s1 = epi.tile([128, 4], F32, name="s1g", tag="s1g")
            for uu in range(4):
                nc.vector.tensor_copy(s1[:, uu:uu + 1],
                                      avt[(b, 0)][uu][:, 256:257])
            inv1 = epi.tile([128, 4], F32, name="inv1", tag="inv1")
            nc.vector.reciprocal(inv1[:], s1[:])
            anps = []
            for uu in range(4):
                anp = epi.tile([128, 512], F32, name="anp", tag="anp", bufs=4)
                nc.vector.tensor_scalar_mul(anp[:, 0:256],
                                            avt[(b, 0)][uu][:, 0:256],
                                            inv1[:, uu:uu + 1])
                anps.append(anp)
            d["anp"] = anps

        def emit_drain1(b):
            # after av(b,1): same for branch 1, then rms accum
            d = EP[b]
            s2g = epi.tile([128, 4], F32, name="s2g", tag="s2g")
            for uu in range(4):
                nc.vector.tensor_copy(s2g[:, uu:uu + 1],
                                      avt[(b, 1)][uu][:, 256:257])
            inv2 = epi.tile([128, 4], F32, name="inv2", tag="inv2")
            nc.vector.reciprocal(inv2[:], s2g[:])
            ss = epi.tile([128, 4], F32, name="ss", tag="ss")
            scr = epi.tile([128, 512], F32, name="scr", tag="scr", bufs=1)
            for uu in range(4):
                anp = d["anp"][uu]
                nc.vector.tensor_scalar_mul(anp[:, 256:512],
                                            avt[(b, 1)][uu][:, 0:256],
                                            inv2[:, uu:uu + 1])
                nc.vector.scalar_tensor_tensor(
                    scr[:], anp[:], 1.0, anp[:], op0=OP.mult, op1=OP.mult,
                    accum_out=ss[:, uu:uu + 1])
            d["ss"] = ss

        def emit_ep_rms(b):
            # Scalar sqrt [128,4]; fake-dep bias ties it into the next
            # batch's sqrt-table block to avoid extra table loads
            d = EP[b]
            rms = epi.tile([128, 4], F32, name="rms", tag="rms")
            nxt = S2Q.get((b + 1, 0))
            if nxt is not None:
                z5 = epi.tile([128, 1], F32, name="z5", tag="z5")
                nc.vector.tensor_scalar(z5[:], nxt[0][:, 0:1], 0.0, 1e-5,
                                        op0=OP.mult, op1=OP.add)
                bias = z5[:]
            else:
                bias = eps5[:]
            nc.scalar.activation(rms[:], d["ss"][:], AF.Sqrt, bias=bias,
                                 scale=1.0 / 256.0)
            d["rms"] = rms

        def emit_ep_rinv(b):
            d = EP[b]
            rinv = epi.tile([128, 4], F32, name="rinv", tag="rinv")
            nc.vector.reciprocal(rinv[:], d["rms"][:])
            d["rinv"] = rinv

        def emit_ep_t(b):
            # GpSimd (SBUF only): t_u = a1n + (-lam)*a2n; rinv folded into
            # the final output copy instead
            d = EP[b]
            ts = []
            for uu in range(4):
                anp = d["anp"][uu]
                t = epi.tile([128, 256], F32, name="tt", tag="tt", bufs=4)
                nc.vector.scalar_tensor_tensor(
                    t[:], anp[:, 256:512], lam_t[:], anp[:, 0:256],
                    op0=OP.mult, op1=OP.add)
                ts.append(t)
            d["t"] = ts
        def emit_ep_transp(b):
            # PE: per unit-pair transpose into one scp tile
            d = EP[b]
            tp = scp.tile([128, 1024], F32, name="scps", tag="sc")
            for pi in range(2):
                for j in range(2):       # unit within pair
                    u = 2 * pi + j
                    base = pi * 512
                    nc.tensor.transpose(tp[:, base + j * 128: base + (j + 1) * 128],
                                        d["t"][u][:, 0:128], ident[:])
                    nc.tensor.transpose(tp[:, base + 256 + j * 128: base + 256 + (j + 1) * 128],
                                        d["t"][u][:, 128:256], ident[:])
            d["tp"] = tp

        def emit_ep_gating(b):
            d = EP[b]
            tp = d["tp"]
            crs = []
            for pi in range(2):
                gcols = slice((b * 4 + 2 * pi) * 128, (b * 4 + 2 * pi + 2) * 128)
                base = pi * 512
                m1 = epi.tile([128, 256], F32, name="m1", tag="m1", bufs=1)
                m2 = epi.tile([128, 256], F32, name="m2", tag="m2", bufs=1)
                m3 = epi.tile([128, 256], F32, name="m3", tag="m3", bufs=1)
                m4 = epi.tile([128, 256], F32, name="m4", tag="m4", bufs=1)
                nc.vector.tensor_mul(m1[:], gT["r"][:, gcols], tp[:, base:base + 256])
                nc.vector.tensor_mul(m2[:], gT["i"][:, gcols], tp[:, base + 256:base + 512])
                nc.vector.tensor_mul(m3[:], gT["i"][:, gcols], tp[:, base:base + 256])
                nc.vector.tensor_mul(m4[:], gT["r"][:, gcols], tp[:, base + 256:base + 512])
                cr = epi.tile([128, 256], F32R, name="cr", tag="cr")
                ci = epi.tile([128, 256], F32R, name="ci", tag="ci")
                nc.gpsimd.tensor_sub(cr[:], m1[:], m2[:])
                nc.gpsimd.tensor_add(ci[:], m3[:], m4[:])
                crs.append((cr, ci))
            d["cr"] = crs
        def emit_ep_outproj(b):
            d = EP[b]
            po = scp.tile([128, 1024], F32, name="scps", tag="sc")
            for uu in range(4):
                cr, ci = d["cr"][uu // 2]
                j = uu % 2
                pos = slice(uu * 256, (uu + 1) * 256)
                nc.tensor.matmul(po[:, pos], cr[:, j * 128:(j + 1) * 128],
                                 W["wo1"][:], start=True, stop=False)
                nc.tensor.matmul(po[:, pos], ci[:, j * 128:(j + 1) * 128],
                                 W["wo2"][:], start=False, stop=True)
            d["po"] = po

        def emit_ep_out(b):
            # Scalar copies PSUM->SBUF, sync DMAs out
            d = EP[b]
            for uu in range(4):
                iu = b * 4 + uu
                ot = osb.tile([128, 256], F32, name="ot", tag="ot")
                nc.scalar.mul(ot[:], d["po"][:, uu * 256:(uu + 1) * 256],
                              d["rinv"][:, uu:uu + 1])
                nc.sync.dma_start(out_d[iu * 128:(iu + 1) * 128, :], ot[:])

        # ---- emission sequence ----
        emit_scores(0, 0)
        s2_00 = emit_sq_s2(0, 0)
        mag_00 = emit_sqrt(0, 0, s2_00)
        emit_exp(0, 0, mag_00)

        emit_scores(0, 1)
        s2_01 = emit_sq_s2(0, 1)
        mag_01 = emit_sqrt(0, 1, s2_01)
        emit_exp(0, 1, mag_01)
        emit_av(0, 0)
        emit_drain0(0)

        # prep head slot 1 here: its v-proj PSUM allocations rotate through
        # the av pool while sc(1,0) runs, instead of stalling batch 1->2
        load_k_side(1, nc.sync)
        kv_prep(1)
        kinctx.close()

        emit_scores(1, 0)
        s2_10 = emit_sq_s2(1, 0)
        mag_10 = emit_sqrt(1, 0, s2_10)
        emit_exp(1, 0, mag_10)
        emit_av(0, 1)

        emit_drain1(0)
        emit_ep_rms(0)
        emit_ep_rinv(0)
        emit_ep_t(0)
        emit_ep_transp(0)
        emit_ep_gating(0)
        emit_ep_outproj(0)

        emit_scores(1, 1)
        s2_11 = emit_sq_s2(1, 1)
        mag_11 = emit_sqrt(1, 1, s2_11)
        emit_ep_out(0)
        emit_exp(1, 1, mag_11)
        emit_av(1, 0)
        emit_drain0(1)

        emit_scores(2, 0)
        s2_20 = emit_sq_s2(2, 0)
        mag_20 = emit_sqrt(2, 0, s2_20)
        emit_exp(2, 0, mag_20)
        emit_av(1, 1)

        emit_drain1(1)
        emit_ep_rms(1)
        emit_ep_rinv(1)
        emit_ep_t(1)
        emit_ep_transp(1)
        emit_ep_gating(1)
        emit_ep_outproj(1)

        emit_scores(2, 1)
        s2_21 = emit_sq_s2(2, 1)
        mag_21 = emit_sqrt(2, 1, s2_21, fine=True)
        emit_ep_out(1)
        emit_exp(2, 1, mag_21, fine=True)
        emit_av(2, 0)
        emit_drain0(2)
        emit_av(2, 1)

        emit_drain1(2)
        emit_ep_rms(2)
        emit_ep_rinv(2)
        emit_ep_t(2)
        emit_ep_transp(2)
        emit_ep_gating(2)
        emit_ep_outproj(2)
        emit_ep_out(2)

    nc.compile()
    return nc


def _get_program():
    if "nc" not in _prog_cache:
        _prog_cache["nc"] = _build_program()
    return _prog_cache["nc"]


def _prep_inputs(inputs):
    f = {k: np.asarray(v, dtype=np.float32) for k, v in inputs.items()}
    lam1 = np.float32(np.exp(np.float32(np.sum(f["lq1"] * f["lk1"]))))
    lam2 = np.float32(np.exp(np.float32(np.sum(f["lq2"] * f["lk2"]))))
    x = np.float32(lam1 - lam2 + np.float32(LAMBDA_INIT))
    lam = np.float32(1.0 / (1.0 + np.exp(-x)))

    wq_rT = f["qw_r"].T.copy()          # [128, 256]
    wq_iT = f["qw_i"].T.copy()
    wk_rT = f["kw_r"].T.copy()          # [128, 128]
    wk_iT = f["kw_i"].T.copy()
    vw_rT = f["vw_r"].T; vw_iT = f["vw_i"].T
    wv1 = np.concatenate([vw_rT, vw_iT], 1).copy()
    wv2 = np.concatenate([-vw_iT, vw_rT], 1).copy()
    wg_rT = f["gw_r"].T.copy(); wg_iT = f["gw_i"].T.copy()
    ow_rT = f["ow_r"].T; ow_iT = f["ow_i"].T
    wo1 = np.concatenate([ow_rT, ow_iT], 1).copy()
    wo2 = np.concatenate([-ow_iT, ow_rT], 1).copy()
    shared = {
        "wq_r": wq_rT, "wq_i": wq_iT, "wq_in": (-wq_iT).copy(),
        "wk_r": wk_rT, "wk_i": wk_iT, "wk_in": (-wk_iT).copy(),
        "wv1": wv1, "wv2": wv2,
        "wg_r": wg_rT, "wg_i": wg_iT, "wg_in": (-wg_iT).copy(),
        "wo1": wo1, "wo2": wo2,
        "lamneg": np.full((128, 1), -lam, np.float32),
    }

    in_maps = []
    for c in range(NCORES):
        units = _core_units(c)
        heads = [units[0][0], units[8][0]]
        m = dict(shared)

        def pack_q(t):
            cols = [t[0, h, q * 128:(q + 1) * 128, :].T for (h, q) in units]
            return np.ascontiguousarray(np.concatenate(cols, 1))
        m["qT_r"] = pack_q(f["q_r"]); m["qT_i"] = pack_q(f["q_i"])
        m["peq_r"] = pack_q(f["pe_q_r"]); m["peq_i"] = pack_q(f["pe_q_i"])

        def pack_k(t):
            return np.ascontiguousarray(
                np.concatenate([t[0, h].T for h in heads], 1))
        m["kT_r"] = pack_k(f["k_r"]); m["kT_i"] = pack_k(f["k_i"])
        m["pek_r"] = pack_k(f["pe_k_r"]); m["pek_i"] = pack_k(f["pe_k_i"])
        m["vT_r"] = pack_k(f["v_r"]); m["vT_i"] = pack_k(f["v_i"])
        in_maps.append(m)
    return in_maps


def _unpack(results):
    out_r = np.zeros((1, H, S, D), np.float32)
    out_i = np.zeros((1, H, S, D), np.float32)
    for c in range(NCORES):
        o = results[c]["out"]
        for u, (h, q) in enumerate(_core_units(c)):
            blk = o[u * 128:(u + 1) * 128]
            out_r[0, h, q * 128:(q + 1) * 128, :] = blk[:, 0:128]
            out_i[0, h, q * 128:(q + 1) * 128, :] = blk[:, 128:256]
    return out_r, out_i


def _run(inputs, trace=False, tmpdir=None):
    nc = _get_program()
    in_maps = _prep_inputs(inputs)
    res = run_bass_kernel_spmd(nc, in_maps, list(range(NCORES)), trace=trace,
                               tmpdir=tmpdir)
    return _unpack(res.results), res


def kernel(**inputs):
    (out_r, out_i), _ = _run(inputs, trace=False)
    return out_r, out_i


# revision 22
# speedup vs baseline: 1.0949x; 1.0174x over previous
"""Trainium2 Bass kernel for ComplexDifferentialAttention.

Sharding: 96 (head, q-tile-of-128) units over 8 cores; each core gets
8 q-tiles of one head (A) + 4 q-tiles of another head (B), so the SPMD
program is identical on every core: 3 batches of 4 q-tiles with
head-slot pattern (A, A, B).

Engine plan (v2):
- PE stream: sc(0,0) | sc(0,1) av(0,0) | sc(1,0) av(0,1) ep(0) |
  sc(1,1) av(1,0) | sc(2,0) av(1,1) ep(1) | sc(2,1) av(2,0) | av(2,1)
  ep(2) -- kept dense so the PE pstate ramps to full clock.
- DVE: squares of score PSUM (drains banks), epilogue accums/scalars.
- GpSimd: half-adds (|s|^2), av drains, u/t builds, half the gating.
- Scalar: ONLY sqrt and exp, in per-branch blocks (sqrt x4-pairs then
  exp x4-pairs) so activation-table reloads drop to ~4/batch; rms and
  output copies slotted into matching table groups.
- Softmax row-sum (ones-column trick) and RMS normalization folded into
  per-query scalars c1 = rinv/sum1, c2 = -lam*rinv/sum2, batched
  [128,4]-wide across the 4 units of a batch.
- et and vp in bf16 (prob/value matmul), everything pre-exp in f32.
"""
import sys, os, math
sys.path.insert(0, '/opt/trn_rl_repo')
import numpy as np
from contextlib import ExitStack

import concourse.bacc as bacc
import concourse.tile as tile
from concourse import mybir
from concourse.bass_utils import run_bass_kernel_spmd
from concourse.masks import make_identity

F32 = mybir.dt.float32
F32R = mybir.dt.float32r
BF16 = mybir.dt.bfloat16
F16 = mybir.dt.float16
AF = mybir.ActivationFunctionType
OP = mybir.AluOpType

D = 128
S = 1024
H = 12
NCORES = 8
NB = 3          # batches per core, 4 units each
SCALE = 1.0 / math.sqrt(D)
LAMBDA_INIT = 0.8 - 0.6 * math.exp(-0.3)

_prog_cache = {}


def _core_units(c):
    """Units for core c: list of (head, qtile). 8 of head A + 4 of head B."""
    k, odd = divmod(c, 2)
    hA = 3 * k + odd          # cores 2k -> 3k ; 2k+1 -> 3k+1
    hB = 3 * k + 2
    qoff = 0 if odd == 0 else 4
    return [(hA, q) for q in range(8)] + [(hB, qoff + q) for q in range(4)]


def _build_program():
    nc = bacc.Bacc("TRN2", target_bir_lowering=False, debug=False,
                   num_devices=NCORES)

    def din(name, shape, dt=F32R):
        return nc.dram_tensor(name, shape, dt, kind="ExternalInput").ap()

    qT_r = din("qT_r", [128, 12 * 128]); qT_i = din("qT_i", [128, 12 * 128])
    peq_r = din("peq_r", [128, 12 * 128], F32)
    peq_i = din("peq_i", [128, 12 * 128], F32)
    kT_r = din("kT_r", [128, 2048]); kT_i = din("kT_i", [128, 2048])
    pek_r = din("pek_r", [128, 2048], F32); pek_i = din("pek_i", [128, 2048], F32)
    vT_r = din("vT_r", [128, 2048]); vT_i = din("vT_i", [128, 2048])
    wq_r = din("wq_r", [128, 256]); wq_i = din("wq_i", [128, 256])
    wq_in = din("wq_in", [128, 256])
    wk_r = din("wk_r", [128, 128]); wk_i = din("wk_i", [128, 128])
    wk_in = din("wk_in", [128, 128])
    wv1 = din("wv1", [128, 256]); wv2 = din("wv2", [128, 256])
    wg_r = din("wg_r", [128, 128]); wg_i = din("wg_i", [128, 128])
    wg_in = din("wg_in", [128, 128])
    wo1 = din("wo1", [128, 256]); wo2 = din("wo2", [128, 256])
    lamneg = din("lamneg", [128, 1], F32)
    out_d = nc.dram_tensor("out", [12 * 128, 256], F32, kind="ExternalOutput").ap()

    with tile.TileContext(nc) as tc, ExitStack() as ctx:
        cst = ctx.enter_context(tc.tile_pool(name="cst", bufs=1))
        qpp = ctx.enter_context(tc.tile_pool(name="qpp", bufs=1))
        kpp = ctx.enter_context(tc.tile_pool(name="kpp", bufs=1))
        vpp = ctx.enter_context(tc.tile_pool(name="vpp", bufs=1))
        hot = ctx.enter_context(tc.tile_pool(name="hot", bufs=1))
        epi = ctx.enter_context(tc.tile_pool(name="epi", bufs=2))
        osb = ctx.enter_context(tc.tile_pool(name="osb", bufs=2))
        scp = ctx.enter_context(tc.tile_pool(name="scp", bufs=2, space="PSUM"))
        avp = ctx.enter_context(tc.tile_pool(name="avp", bufs=4, space="PSUM"))
        # input pools closed right after their last use to free SBUF
        # (LIFO: qin sits on top of kin so it can close first)
        kinctx = ExitStack()
        kin = kinctx.enter_context(tc.tile_pool(name="kin", bufs=1))
        qinctx = ExitStack()
        qin = qinctx.enter_context(tc.tile_pool(name="qin", bufs=1))

        # ---- weights + constants (gpsimd DMA queue: ~25ns issue) ----
        W = {}
        for nm, ap, w in [("wq_r", wq_r, 256), ("wq_i", wq_i, 256),
                          ("wq_in", wq_in, 256), ("wk_r", wk_r, 128),
                          ("wk_i", wk_i, 128), ("wk_in", wk_in, 128),
                          ("wv1", wv1, 256), ("wv2", wv2, 256),
                          ("wg_r", wg_r, 128), ("wg_i", wg_i, 128),
                          ("wg_in", wg_in, 128), ("wo1", wo1, 256),
                          ("wo2", wo2, 256)]:
            t = cst.tile([128, w], F32R, name=f"w_{nm}", tag=f"w_{nm}")
            W[nm] = t
        for nm in ("wq_r", "wq_i", "wq_in"):
            nc.scalar.dma_start(W[nm][:], {"wq_r": wq_r, "wq_i": wq_i,
                                           "wq_in": wq_in}[nm][:])
        lam_t = cst.tile([128, 1], F32)
        ident = cst.tile([128, 128], F32)
        make_identity(nc, ident[:])
        eps8 = cst.tile([128, 1], F32)
        nc.vector.memset(eps8[:], 1e-8)
        eps5 = cst.tile([128, 1], F32)
        nc.vector.memset(eps5[:], 1e-5)

        # ---- q-side inputs (chunked DMAs so matmuls start early) ----
        qtr = qin.tile([128, 1536], F32R, name="qtr", tag="qtr")
        qti = qin.tile([128, 1536], F32R, name="qti", tag="qti")
        pqr = qin.tile([128, 1536], F32, name="pqr", tag="pqr")
        pqi = qin.tile([128, 1536], F32, name="pqi", tag="pqi")
        _wother = [("wk_r", wk_r), ("wk_i", wk_i), ("wk_in", wk_in),
                   ("wv1", wv1), ("wv2", wv2), ("wg_r", wg_r),
                   ("wg_i", wg_i), ("wg_in", wg_in), ("wo1", wo1),
                   ("wo2", wo2)]
        for ch in range(3):
            cs = slice(ch * 512, (ch + 1) * 512)
            nc.sync.dma_start(qtr[:, cs], qT_r[:, cs])
            nc.sync.dma_start(qti[:, cs], qT_i[:, cs])
            nc.scalar.dma_start(pqr[:, cs], peq_r[:, cs])
            nc.scalar.dma_start(pqi[:, cs], peq_i[:, cs])
            for nm, ap in _wother[ch * 4:(ch + 1) * 4]:
                nc.scalar.dma_start(W[nm][:], ap[:])
        for nm, ap in _wother[8:]:
            nc.scalar.dma_start(W[nm][:], ap[:])
        nc.scalar.dma_start(lam_t[:], lamneg[:])

        kint = {}
        for nm in ("ktr", "kti", "pkr", "pki", "vtr", "vti"):
            kint[nm] = kin.tile([128, 1024], F32 if nm in ("pkr", "pki")
                                else F32R, name=nm, tag=nm)

        def load_k_side(hs, q):
            ks_ = slice(hs * 1024, (hs + 1) * 1024)
            for nm, src in [("ktr", kT_r), ("kti", kT_i), ("vtr", vT_r),
                            ("vti", vT_i), ("pkr", pek_r), ("pki", pek_i)]:
                for ch in range(2):
                    cs = slice(ch * 512, (ch + 1) * 512)
                    gs = slice(hs * 1024 + ch * 512, hs * 1024 + (ch + 1) * 512)
                    q.dma_start(kint[nm][:, cs], src[:, gs])

        load_k_side(0, nc.gpsimd)

        # ---- q projection (+pe_q) ----
        qp = {}
        for half in (0, 1):
            hs_ = slice(half * 128, (half + 1) * 128)
            for part in ("r", "i"):
                t = qpp.tile([128, 1536], F32R, name=f"qp{half}{part}",
                             tag=f"qp{half}{part}")
                qp[(half, part)] = t
                for ch in range(3):
                    cs = slice(ch * 512, (ch + 1) * 512)
                    ps = scp.tile([128, 1024], F32, name="scps", tag="sc")
                    if part == "r":
                        nc.tensor.matmul(ps[:, 0:512], W["wq_r"][:, hs_],
                                         qtr[:, cs], start=True, stop=False)
                        nc.tensor.matmul(ps[:, 0:512], W["wq_in"][:, hs_],
                                         qti[:, cs], start=False, stop=True)
                        pe = pqr
                    else:
                        nc.tensor.matmul(ps[:, 0:512], W["wq_i"][:, hs_],
                                         qtr[:, cs], start=True, stop=False)
                        nc.tensor.matmul(ps[:, 0:512], W["wq_r"][:, hs_],
                                         qti[:, cs], start=False, stop=True)
                        pe = pqi
                    nc.vector.tensor_add(t[:, cs], ps[:, 0:512], pe[:, cs])

        # ---- K/V prep for a head slot ----
        kp = {}   # (hs, 'r'|'i'|'in') -> [o=128, k=1024] f32r
        vp = {}   # (hs, chunk) -> [k=128, 258] bf16  ([vp_r | vp_i | 1])

        def kv_prep(hs):
            for part in ("r", "i"):
                t = kpp.tile([128, 1024], F32R, name=f"kp{hs}{part}",
                             tag=f"kp{hs}{part}")
                kp[(hs, part)] = t
                for ch in range(2):
                    cs = slice(ch * 512, (ch + 1) * 512)
                    ps = scp.tile([128, 1024], F32, name="scps", tag="sc")
                    if part == "r":
                        nc.tensor.matmul(ps[:, 0:512], W["wk_r"][:],
                                         kint["ktr"][:, cs], start=True, stop=False)
                        nc.tensor.matmul(ps[:, 0:512], W["wk_in"][:],
                                         kint["kti"][:, cs], start=False, stop=True)
                        pe = kint["pkr"]
                    else:
                        nc.tensor.matmul(ps[:, 0:512], W["wk_i"][:],
                                         kint["ktr"][:, cs], start=True, stop=False)
                        nc.tensor.matmul(ps[:, 0:512], W["wk_r"][:],
                                         kint["kti"][:, cs], start=False, stop=True)
                        pe = kint["pki"]
                    nc.vector.tensor_add(t[:, cs], ps[:, 0:512], pe[:, cs])
            tn = kpp.tile([128, 1024], F32R, name=f"kp{hs}in", tag=f"kp{hs}in")
            kp[(hs, "in")] = tn
            nc.vector.tensor_scalar_mul(tn[:], kp[(hs, "i")][:], -1.0)
            for ch in range(8):
                cs = slice(ch * 128, (ch + 1) * 128)
                ps = avp.tile([128, 258], F32, name="vps", tag="av")
                nc.tensor.matmul(ps[:, 0:256], kint["vtr"][:, cs], W["wv1"][:],
                                 start=True, stop=False)
                nc.tensor.matmul(ps[:, 0:256], kint["vti"][:, cs], W["wv2"][:],
                                 start=False, stop=True)
                vt = vpp.tile([128, 258], BF16, name=f"vp{hs}_{ch}",
                              tag=f"vp{hs}_{ch}")
                vp[(hs, ch)] = vt
                if ch % 2 == 0:
                    nc.vector.tensor_copy(vt[:, 0:256], ps[:, 0:256])
                else:
                    nc.scalar.copy(vt[:, 0:256], ps[:, 0:256])
                nc.vector.memset(vt[:, 256:258], 1.0)

        kv_prep(0)

        # ---- gate projection (scalar copies: before any sqrt/exp) ----
        gT = {}
        for part in ("r", "i"):
            t = qpp.tile([128, 1536], F32, name=f"gT{part}", tag=f"gT{part}")
            gT[part] = t
            for ch in range(3):
                cs = slice(ch * 512, (ch + 1) * 512)
                ps = scp.tile([128, 1024], F32, name="scps", tag="sc")
                if part == "r":
                    nc.tensor.matmul(ps[:, 0:512], W["wg_r"][:], qtr[:, cs],
                                     start=True, stop=False)
                    nc.tensor.matmul(ps[:, 0:512], W["wg_in"][:], qti[:, cs],
                                     start=False, stop=True)
                else:
                    nc.tensor.matmul(ps[:, 0:512], W["wg_i"][:], qtr[:, cs],
                                     start=True, stop=False)
                    nc.tensor.matmul(ps[:, 0:512], W["wg_r"][:], qti[:, cs],
                                     start=False, stop=True)
                nc.scalar.copy(t[:, cs], ps[:, 0:512])

        qinctx.close()   # qtr/qti/pqr/pqi dead; free 24KB before hot tags

        # =========== pipeline ===========
        # branch-step sequence: (b, br) for b in 0..2, br in 0,1
        steps = [(b, br) for b in range(NB) for br in (0, 1)]
        score_ps = {}   # (b, br) -> list of 8 psum tiles
        et = {}         # (b, br, pair) -> [128,1024] bf16
        avt = {}        # (b, br) -> list of 4 av psum tiles [128,258]
        av_sb = {}      # (b, uu) -> br0 av drained [128,257]
        EP = {}         # per-batch epilogue intermediates

        def emit_scores(b, br):
            hs = 0 if b < 2 else 1
            qs = slice(b * 512, (b + 1) * 512)
            tiles = []
            for ch in range(8):
                cs = slice(ch * 128, (ch + 1) * 128)
                ps = scp.tile([128, 1024], F32, name="scps", tag="sc")
                nc.tensor.matmul(ps[:, 0:512], kp[(hs, "r")][:, cs],
                                 qp[(br, "r")][:, qs], start=True, stop=False)
                nc.tensor.matmul(ps[:, 0:512], kp[(hs, "i")][:, cs],
                                 qp[(br, "i")][:, qs], start=False, stop=True)
                nc.tensor.matmul(ps[:, 512:1024], kp[(hs, "r")][:, cs],
                                 qp[(br, "i")][:, qs], start=True, stop=False)
                nc.tensor.matmul(ps[:, 512:1024], kp[(hs, "in")][:, cs],
                                 qp[(br, "r")][:, qs], start=False, stop=True)
                tiles.append(ps)
            score_ps[(b, br)] = tiles

        # square modes per chunk: only ONE PSUM input allowed per op.
        # 'S': Scalar Square drains PSUM directly (cheapest single op)
        # 'D': DVE copy PSUM->SBUF then DVE multiply
        # 'G': DVE copy PSUM->SBUF then GpSimd multiply
        SQMODE = ['S', 'D', 'D', 'D', 'S', 'D', 'G', 'D']
        S2Q = {}

        def emit_sq_s2(b, br):
            # fp16 square/add path (2x DVE rate); x0.5 prescale keeps
            # sq = (x/2)^2 within fp16 range; exp scale doubled later
            s2_tiles = []
            for p in range(2):
                s2 = hot.tile([128, 2048], F16, name="s2", tag="s2", bufs=3)
                s2_tiles.append(s2)
            for ch in range(8):
                ps = score_ps[(b, br)][ch]
                sq = hot.tile([128, 1024], F16, name="sq", tag="sq", bufs=3)
                mode = SQMODE[ch]
                if mode == 'S':
                    nc.scalar.activation(sq[:], ps[:], AF.Square, scale=0.5)
                else:
                    cp = hot.tile([128, 1024], F16, name="cp", tag="cp",
                                  bufs=3)
                    nc.vector.tensor_scalar_mul(cp[:], ps[:], 0.5)
                    if mode == 'D':
                        nc.vector.tensor_mul(sq[:], cp[:], cp[:])
                    else:
                        nc.gpsimd.tensor_mul(sq[:], cp[:], cp[:])
                s2 = s2_tiles[ch // 4]
                slot = slice((ch % 4) * 512, (ch % 4 + 1) * 512)
                if ch % 8 in (0, 3):
                    nc.vector.scalar_tensor_tensor(
                        s2[:, slot], sq[:, 0:512], 1.0, sq[:, 512:1024],
                        op0=OP.mult, op1=OP.add)
                else:
                    nc.gpsimd.tensor_add(s2[:, slot], sq[:, 0:512],
                                         sq[:, 512:1024])
            score_ps.pop((b, br))
            S2Q[(b, br)] = s2_tiles
            return s2_tiles

        def emit_sqrt(b, br, s2_tiles, fine=False):
            # in-place quad sqrt (mag overwrites s2); fine= pair granularity
            if fine:
                for p in range(2):
                    for h in (slice(0, 1024), slice(1024, 2048)):
                        nc.scalar.activation(s2_tiles[p][:, h],
                                             s2_tiles[p][:, h], AF.Sqrt,
                                             bias=eps8[:])
            else:
                for p in range(2):
                    nc.scalar.activation(s2_tiles[p][:], s2_tiles[p][:],
                                         AF.Sqrt, bias=eps8[:])
            return s2_tiles

        def emit_exp(b, br, mags, fine=False):
            # fake dep: exp-q0's bias is produced from mag-q1, forcing both
            # sqrts to run before any exp (keeps table switches at 2/branch);
            # fine= latency-optimized half ops for the pipeline tail
            if fine:
                for p in range(2):
                    e = hot.tile([128, 2048], BF16, name="et", tag="et",
                                 bufs=3)
                    for h in (slice(0, 1024), slice(1024, 2048)):
                        nc.scalar.activation(e[:, h], mags[p][:, h], AF.Exp,
                                             scale=2.0 * SCALE)
                    et[(b, br, p)] = e
                return
            z = epi.tile([128, 1], F32, name="zb", tag="zb")
            nc.vector.tensor_scalar_mul(z[:], mags[1][:, 0:1], 0.0)
            for p in range(2):
                e = hot.tile([128, 2048], BF16, name="et", tag="et", bufs=3)
                nc.scalar.activation(e[:], mags[p][:], AF.Exp,
                                     scale=2.0 * SCALE,
                                     bias=z[:] if p == 0 else 0.0)
                et[(b, br, p)] = e

        def emit_av(b, br):
            hs = 0 if b < 2 else 1
            avs = []
            for uu in range(4):
                av = avp.tile([128, 258], F32, name=f"av{uu}", tag="av")
                avs.append(av)
            for c in range(8):
                e = et[(b, br, c // 4)]
                base = (c % 4) * 512
                for uu in range(4):
                    nc.tensor.matmul(
                        avs[uu][:],
                        e[:, base + uu * 128: base + (uu + 1) * 128],
                        vp[(hs, c)][:],
                        start=(c == 0), stop=(c == 7))
            avt[(b, br)] = avs

        def emit_drain0(b):
            # after av(b,0): gather rowsums, recip, normalize-drain to SBUF
            d = EP.setdefault(b, {})
            s1 = epi.tile([128, 4], F32, name="s1g", tag="s1g")
            for uu in range(4):
                nc.vector.tensor_copy(s1[:, uu:uu + 1],
                                      avt[(b, 0)][uu][:, 256:257])
            inv1 = epi.tile([128, 4], F32, name="inv1", tag="inv1")
            nc.vector.reciprocal(inv1[:], s1[:])
            anps = []
            for uu in range(4):
                anp = epi.tile([128, 512], F32, name="anp", tag="anp", bufs=4)
                nc.vector.tensor_scalar_mul(anp[:, 0:256],
                                            avt[(b, 0)][uu][:, 0:256],
                                            inv1[:, uu:uu + 1])
                anps.append(anp)
            d["anp"] = anps

        def emit_drain1a(b):
            # after av(b,1): gather rowsums + normalize-drain branch 1
            d = EP[b]
            s2g = epi.tile([128, 4], F32, name="s2g", tag="s2g")
            for uu in range(4):
                nc.vector.tensor_copy(s2g[:, uu:uu + 1],
                                      avt[(b, 1)][uu][:, 256:257])
            inv2 = epi.tile([128, 4], F32, name="inv2", tag="inv2")
            nc.vector.reciprocal(inv2[:], s2g[:])
            for uu in range(4):
                nc.vector.tensor_scalar_mul(d["anp"][uu][:, 256:512],
                                            avt[(b, 1)][uu][:, 0:256],
                                            inv2[:, uu:uu + 1])

        def emit_drain1b(b):
            # sum-of-squares accums feed rms; emitted AFTER the t-build so
            # the DVE queue reaches t (and transp can start) first
            d = EP[b]
            ss = epi.tile([128, 4], F32, name="ss", tag="ss")
            scr = epi.tile([128, 512], F32, name="scr", tag="scr", bufs=1)
            for uu in range(4):
                anp = d["anp"][uu]
                nc.vector.scalar_tensor_tensor(
                    scr[:], anp[:], 1.0, anp[:], op0=OP.mult, op1=OP.mult,
                    accum_out=ss[:, uu:uu + 1])
            d["ss"] = ss

        def emit_ep_rms(b):
            # Scalar sqrt [128,4]; fake-dep bias ties it into the next
            # batch's sqrt-table block to avoid extra table loads
            d = EP[b]
            rms = epi.tile([128, 4], F32, name="rms", tag="rms")
            nxt = S2Q.get((b + 1, 0))
            if nxt is not None:
                z5 = epi.tile([128, 1], F32, name="z5", tag="z5")
                nc.vector.tensor_scalar(z5[:], nxt[0][:, 0:1], 0.0, 1e-5,
                                        op0=OP.mult, op1=OP.add)
                bias = z5[:]
            else:
                bias = eps5[:]
            nc.scalar.activation(rms[:], d["ss"][:], AF.Sqrt, bias=bias,
                                 scale=1.0 / 256.0)
            d["rms"] = rms

        def emit_ep_rinv(b):
            d = EP[b]
            rinv = epi.tile([128, 4], F32, name="rinv", tag="rinv")
            nc.vector.reciprocal(rinv[:], d["rms"][:])
            d["rinv"] = rinv

        def emit_ep_t(b):
            # GpSimd (SBUF only): t_u = a1n + (-lam)*a2n; rinv folded into
            # the final output copy instead
            d = EP[b]
            ts = []
            for uu in range(4):
                anp = d["anp"][uu]
                t = epi.tile([128, 256], F32, name="tt", tag="tt", bufs=4)
                nc.vector.scalar_tensor_tensor(
                    t[:], anp[:, 256:512], lam_t[:], anp[:, 0:256],
                    op0=OP.mult, op1=OP.add)
                ts.append(t)
            d["t"] = ts
        def emit_ep_transp(b):
            # PE: per unit-pair transpose into one scp tile
            d = EP[b]
            tp = scp.tile([128, 1024], F32, name="scps", tag="sc")
            for pi in range(2):
                for j in range(2):       # unit within pair
                    u = 2 * pi + j
                    base = pi * 512
                    nc.tensor.transpose(tp[:, base + j * 128: base + (j + 1) * 128],
                                        d["t"][u][:, 0:128], ident[:])
                    nc.tensor.transpose(tp[:, base + 256 + j * 128: base + 256 + (j + 1) * 128],
                                        d["t"][u][:, 128:256], ident[:])
            d["tp"] = tp

        def emit_ep_gating(b):
            d = EP[b]
            tp = d["tp"]
            crs = []
            for pi in range(2):
                gcols = slice((b * 4 + 2 * pi) * 128, (b * 4 + 2 * pi + 2) * 128)
                base = pi * 512
                m1 = epi.tile([128, 256], F32, name="m1", tag="m1", bufs=1)
                m2 = epi.tile([128, 256], F32, name="m2", tag="m2", bufs=1)
                m3 = epi.tile([128, 256], F32, name="m3", tag="m3", bufs=1)
                m4 = epi.tile([128, 256], F32, name="m4", tag="m4", bufs=1)
                nc.vector.tensor_mul(m1[:], gT["r"][:, gcols], tp[:, base:base + 256])
                nc.vector.tensor_mul(m2[:], gT["i"][:, gcols], tp[:, base + 256:base + 512])
                nc.vector.tensor_mul(m3[:], gT["i"][:, gcols], tp[:, base:base + 256])
                nc.vector.tensor_mul(m4[:], gT["r"][:, gcols], tp[:, base + 256:base + 512])
                cr = epi.tile([128, 256], F32R, name="cr", tag="cr")
                ci = epi.tile([128, 256], F32R, name="ci", tag="ci")
                nc.gpsimd.tensor_sub(cr[:], m1[:], m2[:])
                nc.gpsimd.tensor_add(ci[:], m3[:], m4[:])
                crs.append((cr, ci))
            d["cr"] = crs
        def emit_ep_outproj(b):
            d = EP[b]
            po = scp.tile([128, 1024], F32, name="scps", tag="sc")
            for uu in range(4):
                cr, ci = d["cr"][uu // 2]
                j = uu % 2
                pos = slice(uu * 256, (uu + 1) * 256)
                nc.tensor.matmul(po[:, pos], cr[:, j * 128:(j + 1) * 128],
                                 W["wo1"][:], start=True, stop=False)
                nc.tensor.matmul(po[:, pos], ci[:, j * 128:(j + 1) * 128],
                                 W["wo2"][:], start=False, stop=True)
            d["po"] = po

        def emit_ep_out(b):
            # Scalar copies PSUM->SBUF, sync DMAs out
            d = EP[b]
            for uu in range(4):
                iu = b * 4 + uu
                ot = osb.tile([128, 256], F32, name="ot", tag="ot")
                nc.scalar.mul(ot[:], d["po"][:, uu * 256:(uu + 1) * 256],
                              d["rinv"][:, uu:uu + 1])
                nc.sync.dma_start(out_d[iu * 128:(iu + 1) * 128, :], ot[:])

        # ---- emission sequence ----
        emit_scores(0, 0)
        s2_00 = emit_sq_s2(0, 0)
        mag_00 = emit_sqrt(0, 0, s2_00)
        emit_exp(0, 0, mag_00)

        emit_scores(0, 1)
        s2_01 = emit_sq_s2(0, 1)
        mag_01 = emit_sqrt(0, 1, s2_01)
        emit_exp(0, 1, mag_01)
        emit_av(0, 0)
        emit_drain0(0)

        # prep head slot 1 here: its v-proj PSUM allocations rotate through
        # the av pool while sc(1,0) runs, instead of stalling batch 1->2
        load_k_side(1, nc.sync)
        kv_prep(1)
        kinctx.close()

        emit_scores(1, 0)
        s2_10 = emit_sq_s2(1, 0)
        mag_10 = emit_sqrt(1, 0, s2_10)
        emit_exp(1, 0, mag_10)
        emit_av(0, 1)

        emit_drain1a(0)
        emit_ep_t(0)
        emit_drain1b(0)
        emit_ep_transp(0)
        emit_ep_rms(0)
        emit_ep_rinv(0)
        emit_ep_gating(0)
        emit_ep_outproj(0)

        emit_scores(1, 1)
        s2_11 = emit_sq_s2(1, 1)
        mag_11 = emit_sqrt(1, 1, s2_11)
        emit_ep_out(0)
        emit_exp(1, 1, mag_11)
        emit_av(1, 0)
        emit_drain0(1)

        emit_scores(2, 0)
        s2_20 = emit_sq_s2(2, 0)
        mag_20 = emit_sqrt(2, 0, s2_20)
        emit_exp(2, 0, mag_20)
        emit_av(1, 1)

        emit_drain1a(1)
        emit_ep_t(1)
        emit_drain1b(1)
        emit_ep_transp(1)
        emit_ep_rms(1)
        emit_ep_rinv(1)
        emit_ep_gating(1)
        emit_ep_outproj(1)

        emit_scores(2, 1)
        s2_21 = emit_sq_s2(2, 1)
        mag_21 = emit_sqrt(2, 1, s2_21, fine=True)
        emit_ep_out(1)
        emit_exp(2, 1, mag_21, fine=True)
        emit_av(2, 0)
        emit_drain0(2)
        emit_av(2, 1)

        emit_drain1a(2)
        emit_ep_t(2)
        emit_drain1b(2)
        emit_ep_transp(2)
        emit_ep_rms(2)
        emit_ep_rinv(2)
        emit_ep_gating(2)
        emit_ep_outproj(2)
        emit_ep_out(2)

    nc.compile()
    return nc


def _get_program():
    if "nc" not in _prog_cache:
        _prog_cache["nc"] = _build_program()
    return _prog_cache["nc"]


def _prep_inputs(inputs):
    f = {k: np.asarray(v, dtype=np.float32) for k, v in inputs.items()}
    lam1 = np.float32(np.exp(np.float32(np.sum(f["lq1"] * f["lk1"]))))
    lam2 = np.float32(np.exp(np.float32(np.sum(f["lq2"] * f["lk2"]))))
    x = np.float32(lam1 - lam2 + np.float32(LAMBDA_INIT))
    lam = np.float32(1.0 / (1.0 + np.exp(-x)))

    wq_rT = f["qw_r"].T.copy()          # [128, 256]
    wq_iT = f["qw_i"].T.copy()
    wk_rT = f["kw_r"].T.copy()          # [128, 128]
    wk_iT = f["kw_i"].T.copy()
    vw_rT = f["vw_r"].T; vw_iT = f["vw_i"].T
    wv1 = np.concatenate([vw_rT, vw_iT], 1).copy()
    wv2 = np.concatenate([-vw_iT, vw_rT], 1).copy()
    wg_rT = f["gw_r"].T.copy(); wg_iT = f["gw_i"].T.copy()
    ow_rT = f["ow_r"].T; ow_iT = f["ow_i"].T
    wo1 = np.concatenate([ow_rT, ow_iT], 1).copy()
    wo2 = np.concatenate([-ow_iT, ow_rT], 1).copy()
    shared = {
        "wq_r": wq_rT, "wq_i": wq_iT, "wq_in": (-wq_iT).copy(),
        "wk_r": wk_rT, "wk_i": wk_iT, "wk_in": (-wk_iT).copy(),
        "wv1": wv1, "wv2": wv2,
        "wg_r": wg_rT, "wg_i": wg_iT, "wg_in": (-wg_iT).copy(),
        "wo1": wo1, "wo2": wo2,
        "lamneg": np.full((128, 1), -lam, np.float32),
    }

    in_maps = []
    for c in range(NCORES):
        units = _core_units(c)
        heads = [units[0][0], units[8][0]]
        m = dict(shared)

        def pack_q(t):
            cols = [t[0, h, q * 128:(q + 1) * 128, :].T for (h, q) in units]
            return np.ascontiguousarray(np.concatenate(cols, 1))
        m["qT_r"] = pack_q(f["q_r"]); m["qT_i"] = pack_q(f["q_i"])
        m["peq_r"] = pack_q(f["pe_q_r"]); m["peq_i"] = pack_q(f["pe_q_i"])

        def pack_k(t):
            return np.ascontiguousarray(
                np.concatenate([t[0, h].T for h in heads], 1))
        m["kT_r"] = pack_k(f["k_r"]); m["kT_i"] = pack_k(f["k_i"])
        m["pek_r"] = pack_k(f["pe_k_r"]); m["pek_i"] = pack_k(f["pe_k_i"])
        m["vT_r"] = pack_k(f["v_r"]); m["vT_i"] = pack_k(f["v_i"])
        in_maps.append(m)
    return in_maps


def _unpack(results):
    out_r = np.zeros((1, H, S, D), np.float32)
    out_i = np.zeros((1, H, S, D), np.float32)
    for c in range(NCORES):
        o = results[c]["out"]
        for u, (h, q) in enumerate(_core_units(c)):
            blk = o[u * 128:(u + 1) * 128]
            out_r[0, h, q * 128:(q + 1) * 128, :] = blk[:, 0:128]
            out_i[0, h, q * 128:(q + 1) * 128, :] = blk[:, 128:256]
    return out_r, out_i


def _run(inputs, trace=False, tmpdir=None):
    nc = _get_program()
    in_maps = _prep_inputs(inputs)
    res = run_bass_kernel_spmd(nc, in_maps, list(range(NCORES)), trace=trace,
                               tmpdir=tmpdir)
    return _unpack(res.results), res


def kernel(**inputs):
    (out_r, out_i), _ = _run(inputs, trace=False)
    return out_r, out_i
